# revision 1
# baseline (speedup 1.0000x reference)
"""Trainium2 Bass kernel for nn_ContrastByClassCalculator (MoCo-style
per-class-queue contrastive loss).

Math (reference):
    l_pos[n]  = q[n] . k[n]                                  # [N, 1]
    l_neg[n,:] = q[n] @ queue[cls_labels[n]]                 # [N, K]
    logits = concat([l_pos, l_neg], 1) / T                   # [N, 1+K]
    loss = mean_n( -log_softmax(logits)[n, 0] )

Sharding: the queue [C=100, D=128, K=2048] dominates memory traffic
(~105 MB), so we shard it over classes across the 8 cores (13 classes
each, with a 1-class overlap window for the 12-class cores). Each core
computes the full loss rows for the samples whose label falls in its
class range, reduces them to a scalar partial sum on device, and the
host adds the 8 partials and divides by N.

Per-core device program (SPMD, identical structure on all 8 cores):
  - 13 class slots, each padded to 32 sample rows, packed 4 per
    128-partition "group" (4 groups: 4+4+4+1 slots).
  - Per slot: DMA the class's queue slab [128, 2048] to SBUF, then 4
    matmuls (N=512) with the slot's packed q vectors [128, 32] as
    stationary -> PSUM group tile rows 32s..32s+31.
  - Per group: row-max on DVE, fused exp+row-sum on ACT (both read
    PSUM directly), combined with the positive logit (computed on DVE
    from packed q/k rows).
  - Tail: one Ln pass, per-row loss, validity mask, ones-vector matmul
    to reduce over partitions -> scalar partial.

QDT selects the matmul datatype for the l_neg GEMMs:
  - "f32"  : exact fp32 (PE runs 2 half-speed passes, 4 cyc/col)
  - "f32r" : fp32 data, single-pass reduced-precision mode (1 cyc/col)
  - "bf16" : queue+q cast to bf16 on host (halves HBM traffic,
             1 cyc/col).  Loss error stays ~1e-5 relative because the
             row-max subtraction cancels in log-softmax and per-row
             errors average out over N=512.
The positive logits and the whole softmax run in fp32 regardless.
"""

import os

import numpy as np

import concourse.bacc as bacc
import concourse.mybir as mybir
import concourse.tile as tile
from concourse import bass_utils

# Problem constants (hardcoded per contract; kernel.py must be self-contained)
N = 512
D = 128
C = 100
K = 2048
T = 0.07
INV_T = float(1.0 / T)

N_CORES = 8
SLOTS = 13           # class slots per core (4 cores own 13 classes, 4 own 12)
M_PAD = 32           # rows per slot (PE col-group granularity)
GROUP_SLOTS = [(0, 4), (4, 8), (8, 12), (12, 13)]
N_GROUPS = len(GROUP_SLOTS)
# slab DMA chunks: one dispatch costs ~0.7us on the serial HWDGE ring, so
# ship slabs in a few large transfers.  First chunk is a single slab so the
# first matmul can start as early as possible.  Group 3's single slab ships
# BEFORE group 2's chunk (and groups are processed 0,1,3,2) so that when
# the last chunk lands, only ONE group's softmax chain remains on the tail.
DMA_CHUNKS = [(0, 1), (1, 4), (12, 13), (4, 8), (8, 12)]
GROUP_ORDER = [0, 1, 3, 2]
FP32 = mybir.dt.float32
BF16 = mybir.dt.bfloat16
# class range end per core: 4 cores x 13 classes + 4 cores x 12 classes
CLASS_ENDS = [13, 26, 39, 52, 64, 76, 88, 100]

# Matmul/shipping dtype for the l_neg GEMMs.  bf16 halves HBM traffic (the
# memory-bound axis of this problem) and costs ~3.4e-5 relative loss error;
# set BASS_QDT=f32 for the exact (but ~1.5x slower) variant.
QDT = os.environ.get("BASS_QDT", "bf16")  # "bf16" | "f32" | "f32r"

# cpack column layout (fp32 columns); the matmul lhsT ("qt") ships as its
# own tensor so it can carry the matmul dtype end-to-end (walrus requires
# fp32r/bf16 operands to be typed at the producer, not bitcast at use).
QR_OFF = 0                            # [128, 512]  q rows, group-major
KR_OFF = QR_OFF + N_GROUPS * D        # [128, 512]  k rows, group-major
MSK_OFF = KR_OFF + N_GROUPS * D       # [128, 4]    row validity per group
ONE_OFF = MSK_OFF + N_GROUPS          # [128, 1]    all-ones column
CPACK_W = ONE_OFF + 1

# Results of the last hardware run (for test harnesses): BassKernelResults
last_run = None


def _build_nc():
    """Build the single-core SPMD Bass/Tile program.

    Bacc (not raw Bass): its finalize runs generate_event_semaphores,
    which splits multi-semaphore waits to satisfy the TRN2 1-wait-per-
    instruction constraint walrus enforces.
    """
    nc = bacc.Bacc("TRN2")

    mm_dt = {"f32": FP32, "f32r": mybir.dt.float32r, "bf16": BF16}[QDT]

    cpack_h = nc.dram_tensor("cpack", [D, CPACK_W], FP32, kind="ExternalInput")
    qt_h = nc.dram_tensor("qt", [D, SLOTS * M_PAD], mm_dt, kind="ExternalInput")
    slabs_h = nc.dram_tensor("slabs", [SLOTS, D, K], mm_dt, kind="ExternalInput")
    out_h = nc.dram_tensor("out", [1, 1], FP32, kind="ExternalOutput")

    AX = mybir.AxisListType
    AF = mybir.ActivationFunctionType

    with tile.TileContext(nc) as tc:
        with (
            tc.tile_pool(name="consts", bufs=1) as consts,
            tc.tile_pool(name="small", bufs=1) as small,
            tc.tile_pool(name="scr", bufs=2) as scr,
            tc.tile_pool(name="slab", bufs=1) as slab_pool,
            tc.tile_pool(name="esc", bufs=2) as esc_pool,
            tc.tile_pool(name="psum", bufs=2, space="PSUM") as psum_pool,
        ):
            # DMA dispatch order matters (FIFO per HWDGE ring): first slab
            # chunk, then the small qt, then cpack, then remaining chunks
            # alternating across the two rings.
            slab_tiles = {}  # slot -> (tile, col offset)
            for ci, (c0, c1) in enumerate(DMA_CHUNKS):
                st = slab_pool.tile([D, (c1 - c0) * K], mm_dt, tag=f"slab{c0}")
                nc.sync.dma_start(
                    out=st[:], in_=slabs_h[c0:c1].rearrange("n p k -> p n k")
                )
                for t in range(c0, c1):
                    slab_tiles[t] = (st, (t - c0) * K)
                if c0 == 0:
                    qt = consts.tile([D, SLOTS * M_PAD], mm_dt)
                    nc.sync.dma_start(out=qt[:], in_=qt_h[:])
                    # cpack rides early: the positive logits it carries gate
                    # each group's exp bias, and through that the PSUM slot
                    # releases — shipping it late cascades ~5us down the
                    # whole softmax pipeline.
                    cp = consts.tile([D, CPACK_W], FP32)
                    nc.sync.dma_start(out=cp[:], in_=cpack_h[:])

            # Warm the Exp spline table while the first DMAs stream.
            warm = small.tile([1, 1], FP32)
            nc.vector.memset(warm[:], 0.0)
            nc.scalar.activation(out=warm[:], in_=warm[:], func=AF.Exp)

            # Per-row stats, one column per group. Rows beyond a group's
            # active partitions keep the memset values, which yield a row
            # loss of exactly 0 (and are masked anyway).
            lpos = small.tile([128, N_GROUPS], FP32)
            nc.vector.memset(lpos[:], 0.0)
            nbias = small.tile([128, N_GROUPS], FP32)
            nc.vector.memset(nbias[:], 0.0)
            sneg = small.tile([128, N_GROUPS], FP32)
            nc.vector.memset(sneg[:], 0.0)

            for g in GROUP_ORDER:
                t0, t1 = GROUP_SLOTS[g]
                pg = 32 * (t1 - t0)
                col = slice(g, g + 1)

                # positive logit: per-row q.k (multiply then row-reduce)
                ttr = scr.tile([128, D], FP32, tag="ttr")
                nc.vector.tensor_mul(
                    ttr[0:pg],
                    cp[0:pg, QR_OFF + g * D:QR_OFF + (g + 1) * D],
                    cp[0:pg, KR_OFF + g * D:KR_OFF + (g + 1) * D],
                )
                nc.vector.reduce_sum(
                    out=lpos[0:pg, col], in_=ttr[0:pg], axis=AX.X
                )

                gps = psum_pool.tile([128, K], FP32, tag="gps")
                for s in range(t1 - t0):
                    t = t0 + s
                    st, coff = slab_tiles[t]
                    for j in range(K // 512):
                        nc.tensor.matmul(
                            out=gps[32 * s:32 * s + 32, 512 * j:512 * (j + 1)],
                            lhsT=qt[:, M_PAD * t:M_PAD * (t + 1)],
                            rhs=st[:, coff + 512 * j:coff + 512 * (j + 1)],
                            start=True,
                            stop=True,
                            tile_position=(0, 32 * s),
                        )

                # row max over negatives; fold in the positive logit and the
                # -1/T exp-bias scale: nbias = -max(nm,lpos)/T.  The tiny
                # fold runs on the otherwise-idle GpSimd engine so it cannot
                # queue behind another group's 2.3us reduce on DVE (that
                # delay lands directly on the exp critical path at the tail).
                nm = scr.tile([128, 1], FP32, tag="nm")
                nc.vector.reduce_max(out=nm[0:pg], in_=gps[0:pg], axis=AX.X)
                nc.gpsimd.tensor_scalar(
                    out=nbias[0:pg, col],
                    in0=nm[0:pg],
                    scalar1=lpos[0:pg, col],
                    scalar2=-INV_T,
                    op0=mybir.AluOpType.max,
                    op1=mybir.AluOpType.mult,
                )

                # exp((l - rmax)/T) with fused row-sum on ACT
                esc = esc_pool.tile([128, K], FP32, tag="esc")
                nc.scalar.activation(
                    out=esc[0:pg],
                    in_=gps[0:pg],
                    func=AF.Exp,
                    bias=nbias[0:pg, col],
                    scale=INV_T,
                    accum_out=sneg[0:pg, col],
                )

            # Tail, all [128, 4]-wide: the positive-logit exp for every group
            # runs as ONE tiny ACT op: ppos = exp(lpos/T + nbias), then
            # stot = sneg + ppos, row_loss = log(stot) - (lpos/T + nbias),
            # masked, then partition-reduce via ones-vector matmul.
            pprep = small.tile([128, N_GROUPS], FP32)
            nc.vector.scalar_tensor_tensor(
                out=pprep[:], in0=lpos[:], scalar=INV_T, in1=nbias[:],
                op0=mybir.AluOpType.mult, op1=mybir.AluOpType.add,
            )
            ppos = small.tile([128, N_GROUPS], FP32)
            nc.scalar.activation(out=ppos[:], in_=pprep[:], func=AF.Exp)
            stot = small.tile([128, N_GROUPS], FP32)
            nc.vector.tensor_add(stot[:], sneg[:], ppos[:])
            lt = small.tile([128, N_GROUPS], FP32)
            nc.scalar.activation(out=lt[:], in_=stot[:], func=AF.Ln)
            rloss = small.tile([128, N_GROUPS], FP32)
            nc.vector.tensor_sub(rloss[:], lt[:], pprep[:])
            mrl = small.tile([128, N_GROUPS], FP32)
            nc.vector.tensor_mul(mrl[:], rloss[:], cp[:, MSK_OFF:MSK_OFF + N_GROUPS])

            fps = psum_pool.tile([128, K], FP32, tag="gps")
            nc.tensor.matmul(
                out=fps[0:1, 0:N_GROUPS],
                lhsT=cp[:, ONE_OFF:ONE_OFF + 1],
                rhs=mrl[:, 0:N_GROUPS],
                start=True,
                stop=True,
                tile_position=(0, 0),
            )
            osb = small.tile([1, 1], FP32)
            nc.vector.reduce_sum(out=osb[0:1], in_=fps[0:1, 0:N_GROUPS], axis=AX.X)
            nc.sync.dma_start(out=out_h[:], in_=osb[:])

    return nc


def _pack_inputs(q, k, queue, cls_labels):
    """Host-side packing: per-core slab windows + padded per-class q/k rows."""
    import ml_dtypes

    in_maps = []
    for i in range(N_CORES):
        end = CLASS_ENDS[i]
        own_start = CLASS_ENDS[i - 1] if i > 0 else 0
        w0 = end - SLOTS  # slab window start (may include 1 unowned class)

        cpack = np.zeros((D, CPACK_W), dtype=np.float32)
        cpack[:, ONE_OFF] = 1.0
        qt = np.zeros((D, SLOTS * M_PAD), dtype=np.float32)

        for t in range(SLOTS):
            c = w0 + t
            if c < own_start:
                continue  # overlap slot: slab read but no rows assigned
            rows = np.nonzero(cls_labels == c)[0]
            if len(rows) > M_PAD:
                raise ValueError(
                    f"class {c} has {len(rows)} samples > M_PAD={M_PAD}"
                )
            g, s = divmod(t, 4)
            for j, n in enumerate(rows):
                p = 32 * s + j
                qt[:, M_PAD * t + j] = q[n]
                cpack[p, QR_OFF + g * D:QR_OFF + (g + 1) * D] = q[n]
                cpack[p, KR_OFF + g * D:KR_OFF + (g + 1) * D] = k[n]
                cpack[p, MSK_OFF + g] = 1.0

        slabs = np.ascontiguousarray(queue[w0:end], dtype=np.float32)
        if QDT == "bf16":
            slabs = slabs.astype(ml_dtypes.bfloat16)
            qt = qt.astype(ml_dtypes.bfloat16)

        in_maps.append({"cpack": cpack, "qt": qt, "slabs": slabs})
    return in_maps


def kernel(q, k, queue, class_weights, cls_labels):
    global last_run
    q = np.asarray(q, dtype=np.float32)
    k = np.asarray(k, dtype=np.float32)
    queue = np.asarray(queue, dtype=np.float32)
    cls_labels = np.asarray(cls_labels).astype(np.int64)

    in_maps = _pack_inputs(q, k, queue, cls_labels)
    nc = _build_nc()
    if not nc.is_finalized():
        nc.finalize()  # runs Bacc passes: reg alloc + event-semaphore wait split

    trace = bool(os.environ.get("BASS_TRACE"))
    res = bass_utils.run_bass_kernel_spmd(
        nc, in_maps, list(range(N_CORES)), trace=trace
    )
    last_run = res

    partial = sum(float(r["out"][0, 0]) for r in res.results)
    return np.float32(partial / N)



# revision 7
# speedup vs baseline: 1.4056x; 1.4056x over previous
"""Trainium2 Bass kernel for nn_ContrastByClassCalculator (MoCo-style
per-class-queue contrastive loss).

Math (reference):
    l_pos[n]  = q[n] . k[n]                                  # [N, 1]
    l_neg[n,:] = q[n] @ queue[cls_labels[n]]                 # [N, K]
    logits = concat([l_pos, l_neg], 1) / T                   # [N, 1+K]
    loss = mean_n( -log_softmax(logits)[n, 0] )

Strategy (v2):
  * Shard the queue over classes: 13 class slabs per core (8 cores,
    100 classes, 4 cores carry a zero-weight duplicate slab).  The
    class->core assignment is load-balanced on sample count at runtime.
  * Everything ships in fp8 e4m3 (queue slabs + packed q vectors):
    halves HBM traffic vs bf16 at 1.6e-4 relative loss error (the
    row-max subtraction cancels in log-softmax and per-row errors
    average out over N=512).  Slabs are pre-transposed on host to
    [D, SLOTS*K] so every DMA descriptor is one contiguous run per
    partition (13x fewer, 13x larger descriptors than the [C,D,K]
    layout).
  * Dense PSUM packing via interleaved accumulation: each class's
    matmul uses a stationary [D, 128] that is zero except its own
    samples' columns, all accumulating into ONE shared PSUM tile.
    This packs ~62 real sample rows per tile instead of 32-row
    per-class slots, cutting the DVE row-max and ACT exp work ~4x
    (one [128,2048] pass instead of four).
  * Unit D (the last 2, smallest, classes) is folded 4x into a
    [128, 512] tile (k-chunk j -> partition rows 32j..32j+31), so the
    post-last-DMA tail is a 512-wide MAX+EXP instead of 2048-wide.
  * Device outputs only per-row (row_max, sum_exp) partials [128,4];
    the host computes the positive logits, the online-softmax merge,
    the log, and the final mean in float64.  No Ln table load, no
    tail reduction on device.
"""

import os

import numpy as np

import concourse.bacc as bacc
import concourse.mybir as mybir
import concourse.tile as tile
from concourse import bass_utils

# Problem constants (hardcoded per contract; kernel.py must be self-contained)
N = 512
D = 128
C = 100
K = 2048
T = 0.07
INV_T = float(1.0 / T)

N_CORES = 8
SLOTS = 13            # class slabs per core (4 cores: 13 real, 4 cores: 12+dup)
A_SLOTS = 11          # unit A: interleave-accumulated into one [128, K] PSUM tile
D_SLOTS = 2           # unit D: folded 4x into a [128, K/4] PSUM tile
A_ROWS = 128          # unit A sample-row capacity (stationary width)
D_ROWS = 32           # unit D sample-row capacity per fold block
FOLD = 4
KD = K // FOLD        # 512

# slab DMA chunks (slot ranges).  Last two are single slabs so the final
# transfer (gating the tail) is small.
CHUNKS = [(0, 2), (2, 5), (5, 8), (8, 11), (11, 12), (12, 13)]

QW_A = A_SLOTS * A_ROWS          # per-class stationary blocks for unit A
QW = QW_A + D_SLOTS * D_ROWS     # + per-class stationary blocks for unit D

FP32 = mybir.dt.float32
BF16 = mybir.dt.bfloat16
FP8 = mybir.dt.float8e4          # TRN FP8_EXP4 == ml_dtypes.float8_e4m3

# Results of the last hardware run (for test harnesses): BassKernelResults
last_run = None


def _build_nc():
    """Single-core SPMD Bass/Tile program.

    Bacc (not raw Bass): its finalize runs generate_event_semaphores,
    which splits multi-semaphore waits to satisfy the TRN2 1-wait-per-
    instruction constraint walrus enforces.
    """
    nc = bacc.Bacc("TRN2")

    qt_h = nc.dram_tensor("qt", [D, QW], FP8, kind="ExternalInput")
    slabs_h = nc.dram_tensor("slabs", [D, SLOTS * K], FP8, kind="ExternalInput")
    out_h = nc.dram_tensor("out", [128, 4], FP32, kind="ExternalOutput")

    AX = mybir.AxisListType
    AF = mybir.ActivationFunctionType

    with tile.TileContext(nc) as tc:
        with (
            tc.tile_pool(name="consts", bufs=1) as consts,
            tc.tile_pool(name="small", bufs=1) as small,
            tc.tile_pool(name="slab", bufs=1) as slab_pool,
            tc.tile_pool(name="esc", bufs=1) as esc_pool,
            tc.tile_pool(name="psum", bufs=1, space="PSUM") as psum_pool,
        ):
            # qt first (needed by the first LDWEIGHTS), then slab chunks in
            # processing order.  One HWDGE dispatch each; descriptors are one
            # contiguous run per partition.
            qt = consts.tile([D, QW], FP8)
            nc.sync.dma_start(out=qt[:], in_=qt_h[:])
            slab_tiles = {}  # slot -> (tile, col offset)
            for c0, c1 in CHUNKS:
                st = slab_pool.tile([D, (c1 - c0) * K], FP8, tag=f"slab{c0}")
                nc.sync.dma_start(out=st[:], in_=slabs_h[:, c0 * K:c1 * K])
                for t in range(c0, c1):
                    slab_tiles[t] = (st, (t - c0) * K)

            # Warm the Exp spline table while the DMAs stream.
            warm = small.tile([1, 1], FP32)
            nc.vector.memset(warm[:], 0.0)
            nc.scalar.activation(out=warm[:], in_=warm[:], func=AF.Exp)

            # osb columns: 0 = row-max A, 1 = sum-exp A, 2 = row-max D,
            # 3 = sum-exp D.
            osb = small.tile([128, 4], FP32)

            # ---- unit A: 11 classes accumulated into one [128, K] tile ----
            psA = psum_pool.tile([128, K], FP32, tag="psA")
            for a in range(A_SLOTS):
                st, off = slab_tiles[a]
                for h in range(K // 512):
                    nc.tensor.matmul(
                        out=psA[:, 512 * h:512 * (h + 1)],
                        lhsT=qt[:, A_ROWS * a:A_ROWS * (a + 1)],
                        rhs=st[:, off + 512 * h:off + 512 * (h + 1)],
                        start=(a == 0),
                        stop=(a == A_SLOTS - 1),
                        tile_position=(0, 0),
                    )

            nc.vector.reduce_max(out=osb[:, 0:1], in_=psA[:], axis=AX.X)
            biasA = small.tile([128, 1], FP32)
            nc.gpsimd.tensor_scalar_mul(out=biasA[:], in0=osb[:, 0:1], scalar1=-INV_T)
            escA = esc_pool.tile([128, K], BF16, tag="escA")
            nc.scalar.activation(
                out=escA[:],
                in_=psA[:],
                func=AF.Exp,
                bias=biasA[:],
                scale=INV_T,
                accum_out=osb[:, 1:2],
            )

            # ---- unit D: 2 classes folded 4x into a [128, KD] tile ----
            psD = psum_pool.tile([128, KD], FP32, tag="psD")
            for dc in range(D_SLOTS):
                st, off = slab_tiles[A_SLOTS + dc]
                for j in range(FOLD):
                    nc.tensor.matmul(
                        out=psD[D_ROWS * j:D_ROWS * (j + 1), :],
                        lhsT=qt[:, QW_A + D_ROWS * dc:QW_A + D_ROWS * (dc + 1)],
                        rhs=st[:, off + KD * j:off + KD * (j + 1)],
                        start=(dc == 0),
                        stop=(dc == D_SLOTS - 1),
                        tile_position=(0, D_ROWS * j),
                        # fold blocks are partition-disjoint 32-row groups in
                        # one bank; the sim's group tracker is partition-blind
                        skip_group_check=True,
                    )

            nc.vector.reduce_max(out=osb[:, 2:3], in_=psD[:], axis=AX.X)
            biasD = small.tile([128, 1], FP32)
            nc.gpsimd.tensor_scalar_mul(out=biasD[:], in0=osb[:, 2:3], scalar1=-INV_T)
            escD = esc_pool.tile([128, KD], BF16, tag="escD")
            nc.scalar.activation(
                out=escD[:],
                in_=psD[:],
                func=AF.Exp,
                bias=biasD[:],
                scale=INV_T,
                accum_out=osb[:, 3:4],
            )

            nc.sync.dma_start(out=out_h[:], in_=osb[:])

    return nc


def _assign_classes(cls_labels):
    """Load-balanced class->core assignment.

    Returns per-core ordered class lists (length SLOTS; duplicated slab
    for 12-class cores carries no samples) and the dup flags.
    """
    counts = np.bincount(cls_labels, minlength=C)
    caps = [13, 13, 13, 13, 12, 12, 12, 12]
    order = np.argsort(-counts, kind="stable")
    cores = [[] for _ in range(N_CORES)]
    rows = [0] * N_CORES
    for c in order:
        cand = [i for i in range(N_CORES) if len(cores[i]) < caps[i]]
        i = min(cand, key=lambda i: (rows[i], i))
        cores[i].append(int(c))
        rows[i] += int(counts[c])

    plans = []
    for i in range(N_CORES):
        cl = sorted(cores[i], key=lambda c: (counts[c], c))
        d_classes = cl[:D_SLOTS]       # smallest counts -> folded tail unit
        a_classes = cl[D_SLOTS:]
        n_dup = A_SLOTS - len(a_classes)
        # dup slabs ride in unit A with all-zero stationary columns
        slots = a_classes + [a_classes[0]] * n_dup + d_classes
        plans.append({
            "slots": slots,
            "a_classes": a_classes,
            "d_classes": d_classes,
        })
    return plans


def _pack_inputs(q, queue, cls_labels, plans):
    """Per-core fp8 packing: transposed slab windows + masked stationaries.

    Returns (in_maps, row_maps) where row_maps[i] = (a_rows, d_rows):
    a_rows[r] = sample index in unit-A row r; d_rows[r] = sample index in
    unit-D fold-block row r.
    """
    import ml_dtypes

    qf8 = np.ascontiguousarray(q).astype(ml_dtypes.float8_e4m3)
    in_maps, row_maps = [], []
    for i in range(N_CORES):
        p = plans[i]
        qt = np.zeros((D, QW), dtype=ml_dtypes.float8_e4m3)
        a_rows = []
        for a, c in enumerate(p["a_classes"]):
            for n in np.nonzero(cls_labels == c)[0]:
                qt[:, A_ROWS * a + len(a_rows)] = qf8[n]
                a_rows.append(int(n))
        if len(a_rows) > A_ROWS:
            raise ValueError(f"core {i}: {len(a_rows)} unit-A rows > {A_ROWS}")
        d_rows = []
        for dc, c in enumerate(p["d_classes"]):
            for n in np.nonzero(cls_labels == c)[0]:
                qt[:, QW_A + D_ROWS * dc + len(d_rows)] = qf8[n]
                d_rows.append(int(n))
        if len(d_rows) > D_ROWS:
            raise ValueError(f"core {i}: {len(d_rows)} unit-D rows > {D_ROWS}")

        # [SLOTS, D, K] -> [D, SLOTS*K] contiguous per partition
        slabs = np.ascontiguousarray(
            queue[p["slots"]].transpose(1, 0, 2).reshape(D, SLOTS * K)
        ).astype(ml_dtypes.float8_e4m3)

        in_maps.append({"qt": qt, "slabs": slabs})
        row_maps.append((a_rows, d_rows))
    return in_maps, row_maps


def _combine(outs, row_maps, lpos):
    """Host-side float64 merge of per-core device partials -> loss sum."""
    total = 0.0
    for i in range(N_CORES):
        o = np.asarray(outs[i], np.float64)
        a_rows, d_rows = row_maps[i]
        for r, n in enumerate(a_rows):
            m, s = o[r, 0], o[r, 1]
            mt = max(m, lpos[n])
            denom = s * np.exp((m - mt) * INV_T) + np.exp((lpos[n] - mt) * INV_T)
            total += np.log(denom) + (mt - lpos[n]) * INV_T
        for r, n in enumerate(d_rows):
            ms = o[D_ROWS * np.arange(FOLD) + r, 2]
            ss = o[D_ROWS * np.arange(FOLD) + r, 3]
            mt = max(ms.max(), lpos[n])
            denom = (ss * np.exp((ms - mt) * INV_T)).sum() + np.exp(
                (lpos[n] - mt) * INV_T
            )
            total += np.log(denom) + (mt - lpos[n]) * INV_T
    return total


def kernel(q, k, queue, class_weights, cls_labels):
    global last_run
    q = np.asarray(q, dtype=np.float32)
    k = np.asarray(k, dtype=np.float32)
    queue = np.asarray(queue, dtype=np.float32)[:C]
    cls_labels = np.asarray(cls_labels).astype(np.int64)

    plans = _assign_classes(cls_labels)
    in_maps, row_maps = _pack_inputs(q, queue, cls_labels, plans)
    nc = _build_nc()
    if not nc.is_finalized():
        nc.finalize()

    trace = bool(os.environ.get("BASS_TRACE"))
    res = bass_utils.run_bass_kernel_spmd(
        nc, in_maps, list(range(N_CORES)), trace=trace
    )
    last_run = res

    lpos = (q.astype(np.float64) * k.astype(np.float64)).sum(1)
    total = _combine([r["out"] for r in res.results], row_maps, lpos)
    return np.float32(total / N)
